# revision 52
# baseline (speedup 1.0000x reference)
"""Trainium2 Bass kernel v5 for nn_GRNNTransformGated (~1.14ms vs 1.30ms v2).

Key ideas on top of the v2 baseline (fp8 DoubleRow gate matmuls, softmax
z-diff pivot trick, tanh r-gate fold):
  - PER-TREE BIT-REVERSED level storage: children of the node at position
    q live at child-buffer positions q and q+2^j inside the tree's block,
    so every child read is an affine unit-stride-run AP (DVE 2x capable)
    AND every level-output write is plain contiguous. ct columns are
    permuted host-side to match. (Plain evens|odds deinterleaving needed
    pair-strided 16-bit writes, which measure ~4x slow - partial-word RMW.)
  - r and z gate logits grouped in [H,3,MMT] 3-bank PSUM tiles -> one
    3-plane tanh / exp ACT instruction per tile instead of pair+single.
  - denominator fused: den = (f0 + 1) + fS via scalar_tensor_tensor with
    f32 output (kills fp1, denb and the bf16->f32 cast).
  - pair-widened combine: exp/h outputs land in pair tiles; the whole
    combine chain (mlr/mS/m0/a1/a2/fS/den/rcp/out[/out8]) runs 1024-wide
    once per two tiles, halving DVE/ACT per-op fixed overhead (~140ns).
    The LAST two tiles of each level combine singly: the next level's
    early tiles wait on this level's final outputs via the fp8 mirror,
    so tail chain latency matters more than overhead there.
  - u-conv matmuls emitted inside the sweep (t+2 lookahead) instead of a
    level-wide burst (PE is strict FIFO; a DVE-throttled u burst
    head-of-line blocks it), plus next-level u prefetch near level end.
    pu/ph get dedicated single-bank PSUM pools so prefetched u tiles
    never stall the h-matmul rotation (banks: 3+3+1+1).
  - TCH=16 trees/chunk over levels {8,7,6} (chunk output to a full-size
    level-6 buffer); phase B runs levels 5..0 over all trees. Leaf
    (level-9) u tiles for chunk c+1 interleave into chunk c's L7/L6
    sweeps via the next_a hook (e9 WAR is clear after L8).
  - z-gate hH/u contributions as plain BF16 matmuls reading hc/ub
    directly (plain fp8 runs at bf16 speed, so only the (hL,hR) pair is
    worth a DoubleRow cast): kills both per-tile fp8 casts on the DVE and
    two critical-path hops; accumulation groups emit early-ready operands
    first (csl/ub before tanh-dependent rh; child-mirror DRs before hc).
  - single-tile tail levels (L2..L0) split into 2 sub-tiles to keep
    stage_a/stage_b overlap alive at the end of the tree.
  - GPSIMD deliberately idle: offloading mS/fS to it measured +170us
    (shared SBUF port contention + 1.4us/op latency mid-chain).
"""

import sys

for _p in ("/opt/trn_rl_repo", "/root/.axon_site/_ro/trn_rl_repo"):
    if _p not in sys.path:
        sys.path.insert(0, _p)

import numpy as np

B = 1024
L = 10
H = 128
FEAT = 7
NCORES = 8
TPC = B // NCORES          # trees per core = 128
TCH = 16                   # trees per chunk
NCHUNK = TPC // TCH        # 8 chunks
NPC = TPC * (2 ** L - 1)   # nodes per core = 130944
LOFF = [TPC * (2 ** j - 1) for j in range(L + 1)]
LEVEL_SIZES = [B * 2 ** j for j in range(L)]
OFF = np.concatenate([[0], np.cumsum(LEVEL_SIZES)]).astype(int)
INNER = LEVEL_SIZES[:-1]
COFF = np.concatenate([[0], np.cumsum(INNER)]).astype(int)

MMT = 512
SW = 16.0   # fp8 weight scale

_CACHE = {}


def _children_canonical(children):
    for j in range(L - 1):
        n = INNER[j]
        blk = children[COFF[j]:COFF[j + 1]]
        base = 2 * np.arange(n, dtype=np.int64)
        if not (np.array_equal(blk[:, 0], base) and np.array_equal(blk[:, 1], base + 1)):
            return False
    return True


def _numpy_fallback(contents, children, W_u, b_u, W_h, b_h, W_z, b_z, W_r, b_r,
                    conv_w, conv_b):
    w, b = float(conv_w[0]), float(conv_b[0])

    def conv_chain(x):
        for _ in range(3):
            x = np.maximum(w * x + b, 0.0)
        return x

    def sigmoid(x):
        return 1.0 / (1.0 + np.exp(-x))

    emb = None
    for j in reversed(range(L)):
        c = contents[OFF[j]:OFF[j + 1]]
        u = conv_chain(c @ W_u + b_u)
        if j == L - 1:
            emb = u
            continue
        ch = children[COFF[j]:COFF[j + 1]]
        h_L = emb[ch[:, 0]]
        h_R = emb[ch[:, 1]]
        hhu = np.concatenate([h_L, h_R, u], axis=1)
        r = sigmoid(hhu @ W_r + b_r)
        h_H = conv_chain((r * hhu) @ W_h + b_h)
        z = np.concatenate([h_H, hhu], axis=1) @ W_z + b_z
        zs = np.stack([z[:, :H], z[:, H:2 * H], z[:, 2 * H:3 * H], z[:, 3 * H:]], axis=-1)
        zs = zs - zs.max(axis=-1, keepdims=True)
        e = np.exp(zs)
        g = e / e.sum(axis=-1, keepdims=True)
        emb = g[..., 0] * h_H + g[..., 1] * h_L + g[..., 2] * h_R + g[..., 3] * u
    return emb.reshape(B, -1).astype(np.float32)


def _build(cw, cb_, tanh3_ok, exp3_ok):
    from contextlib import ExitStack

    from concourse import bacc, mybir, tile

    f32 = mybir.dt.float32
    bf16 = mybir.dt.bfloat16
    f8 = mybir.dt.float8e4
    AF = mybir.ActivationFunctionType
    OP = mybir.AluOpType
    DR = mybir.MatmulPerfMode.DoubleRow

    A = cw * cw
    C = cw * cb_ + cb_

    nc = bacc.Bacc()

    ct_d = nc.declare_dram_parameter("ct", [FEAT + 1, NPC], bf16, isOutput=False)
    wu_d = nc.declare_dram_parameter("wu", [FEAT + 1, H], bf16, isOutput=False)
    wrlr_d = nc.declare_dram_parameter("wrlr", [H, 3, 2, H], f8, isOutput=False)
    wru_d = nc.declare_dram_parameter("wru", [H, 3, H], bf16, isOutput=False)
    wdlr_d = nc.declare_dram_parameter("wdlr", [H, 3, 2, H], f8, isOutput=False)
    wdh_d = nc.declare_dram_parameter("wdh", [H, 3, H], bf16, isOutput=False)
    wdu_d = nc.declare_dram_parameter("wdu", [H, 3, H], bf16, isOutput=False)
    wh_d = nc.declare_dram_parameter("wh", [H, 3, H], bf16, isOutput=False)
    bv_d = nc.declare_dram_parameter("bvec", [H, 8], f32, isOutput=False)
    id_d = nc.declare_dram_parameter("ident", [H, H], f32, isOutput=False)
    out_d = nc.declare_dram_parameter("out", [TPC, H], f32, isOutput=True)

    with ExitStack() as ctx:
        tc = ctx.enter_context(tile.TileContext(nc))
        wpool = ctx.enter_context(tc.tile_pool(name="wts", bufs=1))
        epool = ctx.enter_context(tc.tile_pool(name="emb", bufs=1))
        ctpool = ctx.enter_context(tc.tile_pool(name="ct", bufs=3))
        upool = ctx.enter_context(tc.tile_pool(name="ub", bufs=4))
        tpool = ctx.enter_context(tc.tile_pool(name="tt", bufs=2))
        rpool = ctx.enter_context(tc.tile_pool(name="rh", bufs=2))
        fpool = ctx.enter_context(tc.tile_pool(name="fl", bufs=2))
        hpool = ctx.enter_context(tc.tile_pool(name="hh", bufs=3))
        spool = ctx.enter_context(tc.tile_pool(name="tmp", bufs=1))
        pool3 = ctx.enter_context(tc.tile_pool(name="pp3", bufs=2, space="PSUM"))
        poolPH = ctx.enter_context(tc.tile_pool(name="ppH", bufs=1, space="PSUM"))
        poolPU = ctx.enter_context(tc.tile_pool(name="ppU", bufs=1, space="PSUM"))

        wu = wpool.tile([FEAT + 1, H], bf16, name="wu", tag="wu")
        wrlr = wpool.tile([H, 3, 2, H], f8, name="wrlr", tag="wrlr")
        wru = wpool.tile([H, 3, H], bf16, name="wru", tag="wru")
        wdlr = wpool.tile([H, 3, 2, H], f8, name="wdlr", tag="wdlr")
        wdh = wpool.tile([H, 3, H], bf16, name="wdh", tag="wdh")
        wdu = wpool.tile([H, 3, H], bf16, name="wdu", tag="wdu")
        wh = wpool.tile([H, 3, H], bf16, name="wh", tag="wh")
        bv = wpool.tile([H, 8], f32, name="bv", tag="bv")
        idt = wpool.tile([H, H], f32, name="idt", tag="idt")
        for t, d in ((wu, wu_d), (wrlr, wrlr_d), (wru, wru_d), (wdlr, wdlr_d),
                     (wdh, wdh_d), (wdu, wdu_d), (wh, wh_d), (bv, bv_d),
                     (idt, id_d)):
            nc.sync.dma_start(t[:], d[:])

        # level buffers, deinterleaved (evens|odds), bf16 + fp8 mirror
        def elvl(name, n, mirror=True):
            eb = epool.tile([H, n], bf16, name=name, tag=name)
            e8 = (epool.tile([H, n], f8, name=name + "_8", tag=name + "_8")
                  if mirror else None)
            return eb, e8

        e9, e9m = elvl("e9", TCH * 512)      # chunk-local leaves
        e8, e8m = elvl("e8", TCH * 256)
        e7, e7m = elvl("e7", TCH * 128)
        em6, em6m = elvl("em6", TPC * 64)    # full, written per chunk
        em5, em5m = elvl("em5", TPC * 32)
        e4, e4m = elvl("e4", TPC * 16)
        e3, e3m = elvl("e3", TPC * 8)
        e2, e2m = elvl("e2", TPC * 4)
        e1, e1m = elvl("e1", TPC * 2)
        e0f = epool.tile([H, TPC], f32, name="e0f", tag="e0f")

        def u_pass(ct_ap, n, ubP, hi):
            """u-conv for one tile; ub written into half `hi` of the pair
            tile ubP so the pair-wide combine reads [H, 2*MMT] contiguous."""
            pu = poolPU.tile([H, MMT], f32, name="pu", tag="pu")
            nc.tensor.matmul(pu[:, :n], wu[:], ct_ap, start=True, stop=True)
            ub = ubP[:, hi, :]
            nc.vector.tensor_scalar(ub[:, :n], pu[:, :n], 0.0, C, OP.max, OP.add)
            return ub

        def stage_a(cbv, cb8v, BL, s, n, ub):
            t0, t1 = s // BL, (s + n) // BL
            cpair8 = cb8v[:, :, t0:t1, :]
            r3 = pool3.tile([H, 3, MMT], f32, name="r3", tag="p3")
            for m in range(3):
                nc.tensor.matmul(r3[:, m, :n], wrlr[:, m], cpair8, start=True,
                                 stop=False, perf_mode=DR, skip_group_check=True)
                nc.tensor.matmul(r3[:, m, :n], wru[:, m], ub[:, :n],
                                 start=False, stop=True, skip_group_check=True)
            tr = tpool.tile([H, 3, MMT], bf16, name="tr", tag="tr")
            if tanh3_ok:
                nc.scalar.activation(tr[:, :, :n], r3[:, :, :n], AF.Tanh,
                                     bias=bv[:, 0:1], scale=0.5 / SW)
            else:
                for m in range(3):
                    nc.scalar.activation(tr[:, m, :n], r3[:, m, :n], AF.Tanh,
                                         bias=bv[:, m:m + 1], scale=0.5 / SW)
            rh = rpool.tile([H, 3, MMT], bf16, name="rh", tag="rh")
            trv = tr[:].rearrange("p m (T b) -> p m T b", b=BL)
            rhv = rh[:].rearrange("p m (T b) -> p m T b", b=BL)
            nc.vector.tensor_tensor(rhv[:, 0, :t1 - t0, :], trv[:, 0, :t1 - t0, :],
                                    cbv[:, 0, t0:t1, :], OP.mult)
            nc.vector.tensor_tensor(rhv[:, 1, :t1 - t0, :], trv[:, 1, :t1 - t0, :],
                                    cbv[:, 1, t0:t1, :], OP.mult)
            nc.vector.tensor_tensor(rh[:, 2, :n], tr[:, 2, :n], ub[:, :n], OP.mult)
            return rh

        def stage_b(cbv, cb8v, BL, s, n, ub, rh, fzP, hcP, hi, next_a,
                    combine):
            t0, t1 = s // BL, (s + n) // BL
            cslL = cbv[:, 0, t0:t1, :]
            cslR = cbv[:, 1, t0:t1, :]
            cpair8 = cb8v[:, :, t0:t1, :]
            # ---- h ----
            ph = poolPH.tile([H, MMT], f32, name="ph", tag="ph")
            nc.tensor.matmul(ph[:, :n], wh[:, 0], cslL, start=True, stop=False)
            nc.tensor.matmul(ph[:, :n], wh[:, 1], cslR, start=False, stop=False)
            nc.tensor.matmul(ph[:, :n], wh[:, 2], ub[:, :n], start=False, stop=False)
            nc.tensor.matmul(ph[:, :n], wh[:, 0], rh[:, 0, :n], start=False, stop=False)
            nc.tensor.matmul(ph[:, :n], wh[:, 1], rh[:, 1, :n], start=False, stop=False)
            nc.tensor.matmul(ph[:, :n], wh[:, 2], rh[:, 2, :n], start=False, stop=True)
            hm = hpool.tile([H, MMT], bf16, name="hm", tag="hm")
            nc.scalar.activation(hm[:, :n], ph[:, :n], AF.Relu, bias=bv[:, 6:7])
            hc = hcP[:, hi, :]
            nc.vector.tensor_scalar_add(hc[:, :n], hm[:, :n], C)
            if next_a is not None:
                next_a()
            # ---- z diff logits (pivot = u gate) ----
            z3 = pool3.tile([H, 3, MMT], f32, name="z3", tag="p3")
            for m in range(3):
                nc.tensor.matmul(z3[:, m, :n], wdlr[:, m], cpair8, start=True,
                                 stop=False, perf_mode=DR, skip_group_check=True)
                nc.tensor.matmul(z3[:, m, :n], wdu[:, m], ub[:, :n],
                                 start=False, stop=False, skip_group_check=True)
            for m in range(3):
                nc.tensor.matmul(z3[:, m, :n], wdh[:, m], hc[:, :n],
                                 start=False, stop=True, skip_group_check=True)
            fz = fzP[:, :, hi, :]
            if exp3_ok:
                nc.scalar.activation(fz[:, :, :n], z3[:, :, :n], AF.Exp,
                                     bias=bv[:, 3:4], scale=1.0 / SW)
            else:
                for m in range(3):
                    nc.scalar.activation(fz[:, m, :n], z3[:, m, :n], AF.Exp,
                                         bias=bv[:, 3 + m:4 + m], scale=1.0 / SW)
            if combine is not None:
                combine()

        def do_combine(cbv, BL, s, W, fzf, hcf, ubf, out_ap, out8_pair):
            """Combine over W nodes starting at tile base s (one tile or a
            pair flattened to [H, *, W] APs):
            out = (u + f0*hH + fL*hL + fR*hR)/(1+f0+fL+fR)."""
            t0, t1 = s // BL, (s + W) // BL
            cslL = cbv[:, 0, t0:t1, :]
            cslR = cbv[:, 1, t0:t1, :]
            mlr = spool.tile([H, 2, 2 * MMT], bf16, name="mlr", tag="mlr")
            mlrv = mlr[:].rearrange("p m (T b) -> p m T b", b=BL)
            fzv = fzf.rearrange("p m (T b) -> p m T b", b=BL)
            nc.vector.tensor_tensor(mlrv[:, 0, :t1 - t0, :], fzv[:, 1, :t1 - t0, :],
                                    cslL, OP.mult)
            nc.vector.tensor_tensor(mlrv[:, 1, :t1 - t0, :], fzv[:, 2, :t1 - t0, :],
                                    cslR, OP.mult)
            mS = spool.tile([H, 2 * MMT], bf16, name="mS", tag="mS")
            nc.vector.tensor_tensor(mS[:, :W], mlr[:, 0, :W], mlr[:, 1, :W], OP.add)
            m0 = spool.tile([H, 2 * MMT], bf16, name="m0", tag="m0")
            nc.vector.tensor_tensor(m0[:, :W], hcf[:, :W], fzf[:, 0, :W], OP.mult)
            a1 = spool.tile([H, 2 * MMT], bf16, name="a1", tag="a1")
            nc.vector.tensor_tensor(a1[:, :W], ubf[:, :W], m0[:, :W], OP.add)
            a2 = spool.tile([H, 2 * MMT], bf16, name="a2", tag="a2")
            nc.vector.tensor_tensor(a2[:, :W], a1[:, :W], mS[:, :W], OP.add)
            fS = spool.tile([H, 2 * MMT], bf16, name="fS", tag="fS")
            nc.vector.tensor_tensor(fS[:, :W], fzf[:, 1, :W], fzf[:, 2, :W], OP.add)
            den = spool.tile([H, 2 * MMT], f32, name="den", tag="den")
            nc.vector.scalar_tensor_tensor(den[:, :W], fzf[:, 0, :W], 1.0,
                                           fS[:, :W], OP.add, OP.add)
            rcp = spool.tile([H, 2 * MMT], f32, name="rcp", tag="rcp")
            nc.vector.reciprocal_approx_fast(rcp[:, :W], den[:, :W])
            nc.vector.tensor_tensor(out_ap, a2[:, :W], rcp[:, :W], OP.mult)
            if out8_pair is not None:
                o8src, o8dst = out8_pair
                nc.scalar.copy(o8dst, o8src)

        class Level:
            """One level of nj nodes with per-tree block size BL=2^j.
            Levels are stored per-tree bit-reversed, so children of the node
            at position q (tree-local) sit at child-buffer positions q and
            q+BL within the tree's 2*BL block; all writes are contiguous.
            prefetch_u() can be called from the PREVIOUS level's tail so the
            first u evictions enqueue ahead of its combine backlog."""

            def __init__(self, nj, BL, ct_base, cb, cb8, ob, ob8, ob_goff,
                         extra_work, tsz=MMT):
                self.nj, self.BL, self.ct_base = nj, BL, ct_base
                self.cb, self.cb8 = cb, cb8
                self.ob, self.ob8, self.ob_goff = ob, ob8, ob_goff
                self.extra_work = extra_work
                self.tiles = [(s, min(tsz, nj - s)) for s in range(0, nj, tsz)]
                self.ctts = {}
                self.us = {}
                self.ubps = {}

            def ct_ap(self, s, n):
                d = (s // 2048) * 2048
                if d not in self.ctts:
                    w = min(2048, self.nj - d)
                    t = ctpool.tile([FEAT + 1, 2048], bf16, name="ctt", tag="ctt")
                    nc.sync.dma_start(t[:, :w],
                                      ct_d[:, self.ct_base + d:self.ct_base + d + w])
                    self.ctts[d] = t
                return self.ctts[d][:, s - d:s - d + n]

            def do_u(self, t):
                if t < len(self.tiles) and t not in self.us:
                    s, n = self.tiles[t]
                    if t % 2 == 0:
                        self.ubps[t // 2] = upool.tile([H, 2, MMT], bf16,
                                                       name="ubP", tag="ubP")
                    self.us[t] = u_pass(self.ct_ap(s, n), n,
                                        self.ubps[t // 2], t % 2)

            def prefetch_u(self):
                self.do_u(0)
                self.do_u(1)

        def run_level(lv, next_lv=None):
            nj, BL = lv.nj, lv.BL
            ob, ob8, ob_goff = lv.ob, lv.ob8, lv.ob_goff
            extra_work = lv.extra_work
            cbv = lv.cb[:].rearrange("p (T two b) -> p two T b", two=2, b=BL)
            cb8v = lv.cb8[:].rearrange("p (T two b) -> p two T b", two=2, b=BL)
            tiles = lv.tiles
            us = lv.us
            ubps = lv.ubps
            do_u = lv.do_u

            rhs_ = {}

            def do_a(t):
                s, n = tiles[t]
                rhs_[t] = stage_a(cbv, cb8v, BL, s, n, us[t])

            do_u(0)
            do_u(1)
            do_a(0)
            fzP = hcP = None
            for t, (s, n) in enumerate(tiles):
                def next_a(t=t):
                    if t + 1 < len(tiles):
                        do_a(t + 1)
                    do_u(t + 2)
                    if t == max(0, len(tiles) - 3) and next_lv is not None:
                        next_lv.prefetch_u()
                    for _ in range(min(4, len(extra_work))):
                        extra_work.pop(0)()
                if t % 2 == 0:
                    fzP = fpool.tile([H, 3, 2, MMT], bf16, name="fzP", tag="fzP")
                    hcP = hpool.tile([H, 2, MMT], bf16, name="hcP", tag="hcP")
                # the last two tiles combine singly: the NEXT level's early
                # tiles depend on this level's final outputs (via the fp8
                # mirror), so halving the tail chain latency beats the pair
                # overhead saving there
                single = t >= len(tiles) - 2
                if single or t % 2 == 1:
                    if single:
                        s0, W = s, n
                        hi = t % 2
                        fzf = fzP[:, :, hi, :]
                        hcf = hcP[:, hi, :]
                        ubf = ubps[t // 2][:, hi, :]
                    else:
                        s0 = tiles[t - 1][0]
                        W = s + n - s0
                        fzf = fzP[:].rearrange("p m two n -> p m (two n)")
                        hcf = hcP[:].rearrange("p two n -> p (two n)")
                        ubf = ubps[t // 2][:].rearrange("p two n -> p (two n)")
                    o_ap = ob[:, ob_goff + s0:ob_goff + s0 + W]
                    o8 = None if ob8 is None else \
                        (o_ap, ob8[:, ob_goff + s0:ob_goff + s0 + W])
                    combine = (lambda s0=s0, W=W, fzf=fzf, hcf=hcf, ubf=ubf,
                               o_ap=o_ap, o8=o8:
                               do_combine(cbv, BL, s0, W, fzf, hcf, ubf,
                                          o_ap, o8))
                else:
                    combine = None
                stage_b(cbv, cb8v, BL, s, n, us[t], rhs_[t],
                        fzP, hcP, t % 2, next_a, combine)

        # leaf (level 9) u-only tiles (bit-reversed ct order -> plain writes)
        def leaf_work(c):
            """Return list of closures, one per leaf tile of chunk c."""
            nleaf = TCH * 512
            base9 = LOFF[9] + c * nleaf
            work = []
            ctts = {}

            def ct_ap(s):
                d = (s // 2048) * 2048
                if d not in ctts:
                    t = ctpool.tile([FEAT + 1, 2048], bf16, name="ctt", tag="ctt")
                    nc.sync.dma_start(t[:], ct_d[:, base9 + d:base9 + d + 2048])
                    ctts[d] = t
                return ctts[d][:, s - d:s - d + MMT]

            def one(s):
                pu = poolPU.tile([H, MMT], f32, name="pu", tag="pu")
                nc.tensor.matmul(pu[:, :MMT], wu[:], ct_ap(s), start=True, stop=True)
                nc.vector.tensor_scalar(e9[:, s:s + MMT], pu[:, :MMT],
                                        0.0, C, OP.max, OP.add)
                nc.scalar.copy(e9m[:, s:s + MMT], e9[:, s:s + MMT])

            for s in range(0, nleaf, MMT):
                work.append(lambda s=s: one(s))
            return work

        # ================= phase A: chunks over levels 8,7,6 =================
        for w in leaf_work(0):
            w()
        # Build the full level schedule. Per chunk, order is
        #   ..., L8(c+1), L6(c), L7(c+1), ...
        # L6(c) and L8(c+1) touch disjoint buffers, so each short level sits
        # between independent work and level tails (whose final outputs gate
        # the consumer's early tiles) stay hidden. Leaf tiles of chunk c+1
        # drain through L7(c)'s and L8(c+1)'s next_a slots.
        def A8(c, work):
            return Level(TCH * 256, 256, LOFF[8] + c * TCH * 256,
                         e9, e9m, e8, e8m, 0, work)

        def A7(c, work):
            return Level(TCH * 128, 128, LOFF[7] + c * TCH * 128,
                         e8, e8m, e7, e7m, 0, work)

        def A6(c):
            return Level(TCH * 64, 64, LOFF[6] + c * TCH * 64,
                         e7, e7m, em6, em6m, c * TCH * 64, [])

        levels = []
        pend0 = leaf_work(0)
        for w in pend0[:4]:
            w()
        del pend0[:4]
        pend = leaf_work(1)
        levels.append(A8(0, pend0))
        levels.append(A7(0, pend))
        for c in range(NCHUNK - 1):
            nxt = leaf_work(c + 2) if c + 2 < NCHUNK else []
            levels.append(A8(c + 1, pend))
            levels.append(A6(c))
            levels.append(A7(c + 1, nxt))
            pend = nxt
        levels.append(A6(NCHUNK - 1))
        chain = [(em6, em6m, em5, em5m), (em5, em5m, e4, e4m),
                 (e4, e4m, e3, e3m), (e3, e3m, e2, e2m), (e2, e2m, e1, e1m)]
        for j, (cbuf, cb8, obuf, ob8) in zip(range(5, 0, -1), chain):
            tsz = MMT if TPC * 2 ** j > MMT else TPC * 2 ** (j - 1)
            levels.append(Level(TPC * 2 ** j, 2 ** j, LOFF[j],
                                cbuf, cb8, obuf, ob8, 0, [], tsz))
        levels.append(Level(TPC, 1, LOFF[0], e1, e1m, e0f, None, 0, [],
                            TPC // 2))

        for i, lv in enumerate(levels):
            nxt = levels[i + 1] if i + 1 < len(levels) else None
            run_level(lv, nxt)
            # safety: any leaf tiles not drained by the slots run now
            while lv.extra_work:
                lv.extra_work.pop(0)()

        # ================= output transpose + store =================
        pt = poolPH.tile([H, MMT], f32, name="ptr", tag="ph")
        nc.tensor.matmul(pt[:, :H], e0f[:], idt[:], is_transpose=True,
                         start=True, stop=True)
        osb = spool.tile([H, MMT], f32, name="osb", tag="osb")
        nc.vector.tensor_copy(osb[:, :H], pt[:, :H])
        nc.sync.dma_start(out_d[:], osb[:, :H])

    nc.compile()
    if not nc.is_finalized():
        nc.finalize()
    return nc


def _prepare(inputs):
    import ml_dtypes

    bf = ml_dtypes.bfloat16
    f8 = ml_dtypes.float8_e4m3

    contents = np.ascontiguousarray(np.asarray(inputs["contents"], np.float32))
    W_u = np.asarray(inputs["W_u"], np.float32)
    b_u = np.asarray(inputs["b_u"], np.float32)
    W_h = np.asarray(inputs["W_h"], np.float32)
    b_h = np.asarray(inputs["b_h"], np.float32)
    W_z = np.asarray(inputs["W_z"], np.float32)
    b_z = np.asarray(inputs["b_z"], np.float32)
    W_r = np.asarray(inputs["W_r"], np.float32)
    b_r = np.asarray(inputs["b_r"], np.float32)
    cw = float(np.asarray(inputs["conv_w"]).reshape(-1)[0])
    cb_ = float(np.asarray(inputs["conv_b"]).reshape(-1)[0])
    A = cw * cw
    C = cw * cb_ + cb_

    # per-core feature-major contents + ones row, level-major columns;
    # within each tree's level block, nodes in bit-reversed order so the
    # on-chip level buffers (position order) give contiguous children reads
    def bitrev_perm(j):
        p = np.arange(2 ** j, dtype=np.int64)
        r = np.zeros_like(p)
        for b in range(j):
            r = (r << 1) | ((p >> b) & 1)
        return r

    cts = np.empty((NCORES, FEAT + 1, NPC), np.float32)
    cts[:, FEAT, :] = 1.0
    col = 0
    for j in range(L):
        nn = 2 ** j
        n = TPC * nn
        blk = contents[OFF[j]:OFF[j + 1]].reshape(NCORES, TPC, nn, FEAT)
        blk = blk[:, :, bitrev_perm(j), :].reshape(NCORES, n, FEAT)
        cts[:, :FEAT, col:col + n] = blk.transpose(0, 2, 1)
        col += n

    # u-conv weights: pu = (A*cw*W_u)^T c + A*(cw*b_u + cb)
    wu = np.empty((FEAT + 1, H), np.float32)
    wu[:FEAT] = A * cw * W_u
    wu[FEAT] = A * (cw * b_u + cb_)

    # r weights (x SW, fp8): blocks rows 0:H=hL, H:2H=hR, 2H:3H=u
    wrlr = np.empty((H, 3, 2, H), np.float32)
    wru = np.empty((H, 3, H), np.float32)
    for m in range(3):
        blk = slice(m * H, (m + 1) * H)
        wrlr[:, m, 0, :] = SW * W_r[0:H, blk]
        wrlr[:, m, 1, :] = SW * W_r[H:2 * H, blk]
        wru[:, m, :] = SW * W_r[2 * H:3 * H, blk]

    # z diff weights: Wd_m = W_z[:, m] - W_z[:, u-gate], rows 0:H=hH, H:2H=hL,
    # 2H:3H=hR, 3H:4H=u
    wdlr = np.empty((H, 3, 2, H), np.float32)
    wdh = np.empty((H, 3, H), np.float32)
    wdu = np.empty((H, 3, H), np.float32)
    bd = np.empty((3, H), np.float32)
    for m in range(3):
        Wd = W_z[:, m * H:(m + 1) * H] - W_z[:, 3 * H:4 * H]
        bd[m] = b_z[m * H:(m + 1) * H] - b_z[3 * H:4 * H]
        wdlr[:, m, 0, :] = SW * Wd[H:2 * H]
        wdlr[:, m, 1, :] = SW * Wd[2 * H:3 * H]
        wdh[:, m, :] = SW * Wd[0:H]
        wdu[:, m, :] = SW * Wd[3 * H:4 * H]

    # h weights: ph = (0.5*A*cw*W_h)^T rh'  with rh' = (t+1)*hhu
    wh = np.ascontiguousarray((0.5 * A * cw * W_h).reshape(3, H, H).transpose(1, 0, 2))

    bvec = np.zeros((H, 8), np.float32)
    bvec[:, 0] = 0.5 * b_r[0:H]
    bvec[:, 1] = 0.5 * b_r[H:2 * H]
    bvec[:, 2] = 0.5 * b_r[2 * H:3 * H]
    bvec[:, 3] = bd[0]
    bvec[:, 4] = bd[1]
    bvec[:, 5] = bd[2]
    bvec[:, 6] = A * (cw * b_h + cb_)

    tanh3_ok = bool(np.array_equal(bvec[:, 0], bvec[:, 1])
                    and np.array_equal(bvec[:, 1], bvec[:, 2]))
    exp3_ok = bool(np.array_equal(bvec[:, 3], bvec[:, 4])
                   and np.array_equal(bvec[:, 4], bvec[:, 5]))

    common = {
        "wu": np.ascontiguousarray(wu).astype(bf),
        "wrlr": np.ascontiguousarray(wrlr).astype(f8),
        "wru": np.ascontiguousarray(wru).astype(bf),
        "wdlr": np.ascontiguousarray(wdlr).astype(f8),
        "wdh": np.ascontiguousarray(wdh).astype(bf),
        "wdu": np.ascontiguousarray(wdu).astype(bf),
        "wh": wh.astype(bf),
        "bvec": bvec,
        "ident": np.eye(H, dtype=np.float32),
    }
    in_maps = [dict(common, ct=np.ascontiguousarray(cts[c]).astype(bf))
               for c in range(NCORES)]
    return in_maps, tanh3_ok, exp3_ok


def kernel(**inputs):
    children = np.asarray(inputs["children"])
    cw = float(np.asarray(inputs["conv_w"]).reshape(-1)[0])
    cb_ = float(np.asarray(inputs["conv_b"]).reshape(-1)[0])
    collapsible = (cw >= 0.0) and (cb_ >= 0.0)
    if not _children_canonical(children) or not collapsible:
        args = {k: np.asarray(v) for k, v in inputs.items()}
        return _numpy_fallback(**args)

    from concourse.bass_utils import run_bass_kernel_spmd

    in_maps, tanh3_ok, exp3_ok = _prepare(inputs)
    key = (cw, cb_, tanh3_ok, exp3_ok)
    if key not in _CACHE:
        _CACHE[key] = _build(cw, cb_, tanh3_ok, exp3_ok)
    nc = _CACHE[key]

    res = run_bass_kernel_spmd(nc, in_maps, list(range(NCORES)))
    outs = [res.results[c]["out"] for c in range(NCORES)]
    return np.ascontiguousarray(np.concatenate(outs, axis=0).astype(np.float32))


if __name__ == "__main__":
    print("kernel_v3 module loaded")


# revision 55
# speedup vs baseline: 1.0091x; 1.0091x over previous
"""Trainium2 Bass kernel v5 for nn_GRNNTransformGated (~1.14ms vs 1.30ms v2).

Key ideas on top of the v2 baseline (fp8 DoubleRow gate matmuls, softmax
z-diff pivot trick, tanh r-gate fold):
  - PER-TREE BIT-REVERSED level storage: children of the node at position
    q live at child-buffer positions q and q+2^j inside the tree's block,
    so every child read is an affine unit-stride-run AP (DVE 2x capable)
    AND every level-output write is plain contiguous. ct columns are
    permuted host-side to match. (Plain evens|odds deinterleaving needed
    pair-strided 16-bit writes, which measure ~4x slow - partial-word RMW.)
  - r and z gate logits grouped in [H,3,MMT] 3-bank PSUM tiles -> one
    3-plane tanh / exp ACT instruction per tile instead of pair+single.
  - denominator fused: den = (f0 + 1) + fS via scalar_tensor_tensor with
    f32 output (kills fp1, denb and the bf16->f32 cast).
  - pair-widened combine: exp/h outputs land in pair tiles; the whole
    combine chain (mlr/mS/m0/a1/a2/fS/den/rcp/out[/out8]) runs 1024-wide
    once per two tiles, halving DVE/ACT per-op fixed overhead (~140ns).
    The LAST two tiles of each level combine singly: the next level's
    early tiles wait on this level's final outputs via the fp8 mirror,
    so tail chain latency matters more than overhead there.
  - u-conv matmuls emitted inside the sweep (t+2 lookahead) instead of a
    level-wide burst (PE is strict FIFO; a DVE-throttled u burst
    head-of-line blocks it), plus next-level u prefetch near level end.
    pu/ph get dedicated single-bank PSUM pools so prefetched u tiles
    never stall the h-matmul rotation (banks: 3+3+1+1).
  - TCH=16 trees/chunk over levels {8,7,6} (chunk output to a full-size
    level-6 buffer); phase B runs levels 5..0 over all trees. Leaf
    (level-9) u tiles for chunk c+1 interleave into chunk c's L7/L6
    sweeps via the next_a hook (e9 WAR is clear after L8).
  - z-gate hH/u contributions as plain BF16 matmuls reading hc/ub
    directly (plain fp8 runs at bf16 speed, so only the (hL,hR) pair is
    worth a DoubleRow cast): kills both per-tile fp8 casts on the DVE and
    two critical-path hops; accumulation groups emit early-ready operands
    first (csl/ub before tanh-dependent rh; child-mirror DRs before hc).
  - single-tile tail levels (L2..L0) split into 2 sub-tiles to keep
    stage_a/stage_b overlap alive at the end of the tree.
  - GPSIMD deliberately idle: offloading mS/fS to it measured +170us
    (shared SBUF port contention + 1.4us/op latency mid-chain).
"""

import sys

for _p in ("/opt/trn_rl_repo", "/root/.axon_site/_ro/trn_rl_repo"):
    if _p not in sys.path:
        sys.path.insert(0, _p)

import numpy as np

B = 1024
L = 10
H = 128
FEAT = 7
NCORES = 8
TPC = B // NCORES          # trees per core = 128
TCH = 16                   # trees per chunk
NCHUNK = TPC // TCH        # 8 chunks
NPC = TPC * (2 ** L - 1)   # nodes per core = 130944
LOFF = [TPC * (2 ** j - 1) for j in range(L + 1)]
LEVEL_SIZES = [B * 2 ** j for j in range(L)]
OFF = np.concatenate([[0], np.cumsum(LEVEL_SIZES)]).astype(int)
INNER = LEVEL_SIZES[:-1]
COFF = np.concatenate([[0], np.cumsum(INNER)]).astype(int)

MMT = 512
SW = 16.0   # fp8 weight scale

_CACHE = {}


def _children_canonical(children):
    for j in range(L - 1):
        n = INNER[j]
        blk = children[COFF[j]:COFF[j + 1]]
        base = 2 * np.arange(n, dtype=np.int64)
        if not (np.array_equal(blk[:, 0], base) and np.array_equal(blk[:, 1], base + 1)):
            return False
    return True


def _numpy_fallback(contents, children, W_u, b_u, W_h, b_h, W_z, b_z, W_r, b_r,
                    conv_w, conv_b):
    w, b = float(conv_w[0]), float(conv_b[0])

    def conv_chain(x):
        for _ in range(3):
            x = np.maximum(w * x + b, 0.0)
        return x

    def sigmoid(x):
        return 1.0 / (1.0 + np.exp(-x))

    emb = None
    for j in reversed(range(L)):
        c = contents[OFF[j]:OFF[j + 1]]
        u = conv_chain(c @ W_u + b_u)
        if j == L - 1:
            emb = u
            continue
        ch = children[COFF[j]:COFF[j + 1]]
        h_L = emb[ch[:, 0]]
        h_R = emb[ch[:, 1]]
        hhu = np.concatenate([h_L, h_R, u], axis=1)
        r = sigmoid(hhu @ W_r + b_r)
        h_H = conv_chain((r * hhu) @ W_h + b_h)
        z = np.concatenate([h_H, hhu], axis=1) @ W_z + b_z
        zs = np.stack([z[:, :H], z[:, H:2 * H], z[:, 2 * H:3 * H], z[:, 3 * H:]], axis=-1)
        zs = zs - zs.max(axis=-1, keepdims=True)
        e = np.exp(zs)
        g = e / e.sum(axis=-1, keepdims=True)
        emb = g[..., 0] * h_H + g[..., 1] * h_L + g[..., 2] * h_R + g[..., 3] * u
    return emb.reshape(B, -1).astype(np.float32)


def _build(cw, cb_, tanh3_ok, exp3_ok):
    from contextlib import ExitStack

    from concourse import bacc, mybir, tile

    f32 = mybir.dt.float32
    bf16 = mybir.dt.bfloat16
    f8 = mybir.dt.float8e4
    AF = mybir.ActivationFunctionType
    OP = mybir.AluOpType
    DR = mybir.MatmulPerfMode.DoubleRow

    A = cw * cw
    C = cw * cb_ + cb_

    nc = bacc.Bacc()

    ct_d = nc.declare_dram_parameter("ct", [FEAT + 1, NPC], bf16, isOutput=False)
    wu_d = nc.declare_dram_parameter("wu", [FEAT + 1, H], bf16, isOutput=False)
    wrlr_d = nc.declare_dram_parameter("wrlr", [H, 3, 2, H], f8, isOutput=False)
    wru_d = nc.declare_dram_parameter("wru", [H, 3, H], bf16, isOutput=False)
    wdlr_d = nc.declare_dram_parameter("wdlr", [H, 3, 2, H], f8, isOutput=False)
    wdh_d = nc.declare_dram_parameter("wdh", [H, 3, H], bf16, isOutput=False)
    wdu_d = nc.declare_dram_parameter("wdu", [H, 3, H], bf16, isOutput=False)
    wh_d = nc.declare_dram_parameter("wh", [H, 3, H], bf16, isOutput=False)
    bv_d = nc.declare_dram_parameter("bvec", [H, 8], f32, isOutput=False)
    id_d = nc.declare_dram_parameter("ident", [H, H], f32, isOutput=False)
    out_d = nc.declare_dram_parameter("out", [TPC, H], f32, isOutput=True)

    with ExitStack() as ctx:
        tc = ctx.enter_context(tile.TileContext(nc))
        wpool = ctx.enter_context(tc.tile_pool(name="wts", bufs=1))
        epool = ctx.enter_context(tc.tile_pool(name="emb", bufs=1))
        ctpool = ctx.enter_context(tc.tile_pool(name="ct", bufs=3))
        upool = ctx.enter_context(tc.tile_pool(name="ub", bufs=4))
        tpool = ctx.enter_context(tc.tile_pool(name="tt", bufs=2))
        rpool = ctx.enter_context(tc.tile_pool(name="rh", bufs=2))
        fpool = ctx.enter_context(tc.tile_pool(name="fl", bufs=2))
        hpool = ctx.enter_context(tc.tile_pool(name="hh", bufs=3))
        spool = ctx.enter_context(tc.tile_pool(name="tmp", bufs=1))
        pool3 = ctx.enter_context(tc.tile_pool(name="pp3", bufs=2, space="PSUM"))
        poolPH = ctx.enter_context(tc.tile_pool(name="ppH", bufs=1, space="PSUM"))
        poolPU = ctx.enter_context(tc.tile_pool(name="ppU", bufs=1, space="PSUM"))

        wu = wpool.tile([FEAT + 1, H], bf16, name="wu", tag="wu")
        wrlr = wpool.tile([H, 3, 2, H], f8, name="wrlr", tag="wrlr")
        wru = wpool.tile([H, 3, H], bf16, name="wru", tag="wru")
        wdlr = wpool.tile([H, 3, 2, H], f8, name="wdlr", tag="wdlr")
        wdh = wpool.tile([H, 3, H], bf16, name="wdh", tag="wdh")
        wdu = wpool.tile([H, 3, H], bf16, name="wdu", tag="wdu")
        wh = wpool.tile([H, 3, H], bf16, name="wh", tag="wh")
        bv = wpool.tile([H, 8], f32, name="bv", tag="bv")
        idt = wpool.tile([H, H], f32, name="idt", tag="idt")
        for t, d in ((wu, wu_d), (wrlr, wrlr_d), (wru, wru_d), (wdlr, wdlr_d),
                     (wdh, wdh_d), (wdu, wdu_d), (wh, wh_d), (bv, bv_d),
                     (idt, id_d)):
            nc.sync.dma_start(t[:], d[:])

        # level buffers, deinterleaved (evens|odds), bf16 + fp8 mirror
        def elvl(name, n, mirror=True):
            eb = epool.tile([H, n], bf16, name=name, tag=name)
            e8 = (epool.tile([H, n], f8, name=name + "_8", tag=name + "_8")
                  if mirror else None)
            return eb, e8

        e9, e9m = elvl("e9", TCH * 512)      # chunk-local leaves
        e8, e8m = elvl("e8", TCH * 256)
        e7, e7m = elvl("e7", TCH * 128)
        em6, em6m = elvl("em6", TPC * 64)    # full, written per chunk
        em5, em5m = elvl("em5", TPC * 32)
        e4, e4m = elvl("e4", TPC * 16)
        e3, e3m = elvl("e3", TPC * 8)
        e2, e2m = elvl("e2", TPC * 4)
        e1, e1m = elvl("e1", TPC * 2)
        e0f = epool.tile([H, TPC], f32, name="e0f", tag="e0f")

        def u_pass(ct_ap, n, ubP, hi):
            """u-conv for one tile; ub written into half `hi` of the pair
            tile ubP so the pair-wide combine reads [H, 2*MMT] contiguous."""
            pu = poolPU.tile([H, MMT], f32, name="pu", tag="pu")
            nc.tensor.matmul(pu[:, :n], wu[:], ct_ap, start=True, stop=True)
            ub = ubP[:, hi, :]
            nc.vector.tensor_scalar(ub[:, :n], pu[:, :n], 0.0, C, OP.max, OP.add)
            return ub

        def stage_a(cbv, cb8v, BL, s, n, ub):
            t0, t1 = s // BL, (s + n) // BL
            cpair8 = cb8v[:, :, t0:t1, :]
            r3 = pool3.tile([H, 3, MMT], f32, name="r3", tag="p3")
            for m in range(3):
                nc.tensor.matmul(r3[:, m, :n], wrlr[:, m], cpair8, start=True,
                                 stop=False, perf_mode=DR, skip_group_check=True)
                nc.tensor.matmul(r3[:, m, :n], wru[:, m], ub[:, :n],
                                 start=False, stop=True, skip_group_check=True)
            tr = tpool.tile([H, 3, MMT], bf16, name="tr", tag="tr")
            if tanh3_ok:
                nc.scalar.activation(tr[:, :, :n], r3[:, :, :n], AF.Tanh,
                                     bias=bv[:, 0:1], scale=0.5 / SW)
            else:
                for m in range(3):
                    nc.scalar.activation(tr[:, m, :n], r3[:, m, :n], AF.Tanh,
                                         bias=bv[:, m:m + 1], scale=0.5 / SW)
            rh = rpool.tile([H, 3, MMT], bf16, name="rh", tag="rh")
            trv = tr[:].rearrange("p m (T b) -> p m T b", b=BL)
            rhv = rh[:].rearrange("p m (T b) -> p m T b", b=BL)
            nc.vector.tensor_tensor(rhv[:, 0, :t1 - t0, :], trv[:, 0, :t1 - t0, :],
                                    cbv[:, 0, t0:t1, :], OP.mult)
            nc.vector.tensor_tensor(rhv[:, 1, :t1 - t0, :], trv[:, 1, :t1 - t0, :],
                                    cbv[:, 1, t0:t1, :], OP.mult)
            nc.vector.tensor_tensor(rh[:, 2, :n], tr[:, 2, :n], ub[:, :n], OP.mult)
            return rh

        def stage_b(cbv, cb8v, BL, s, n, ub, rh, fzP, hcP, hi, next_a,
                    combine):
            t0, t1 = s // BL, (s + n) // BL
            cslL = cbv[:, 0, t0:t1, :]
            cslR = cbv[:, 1, t0:t1, :]
            cpair8 = cb8v[:, :, t0:t1, :]
            # ---- h ----
            ph = poolPH.tile([H, MMT], f32, name="ph", tag="ph")
            nc.tensor.matmul(ph[:, :n], wh[:, 0], cslL, start=True, stop=False)
            nc.tensor.matmul(ph[:, :n], wh[:, 1], cslR, start=False, stop=False)
            nc.tensor.matmul(ph[:, :n], wh[:, 2], ub[:, :n], start=False, stop=False)
            nc.tensor.matmul(ph[:, :n], wh[:, 0], rh[:, 0, :n], start=False, stop=False)
            nc.tensor.matmul(ph[:, :n], wh[:, 1], rh[:, 1, :n], start=False, stop=False)
            nc.tensor.matmul(ph[:, :n], wh[:, 2], rh[:, 2, :n], start=False, stop=True)
            hm = hpool.tile([H, MMT], bf16, name="hm", tag="hm")
            nc.scalar.activation(hm[:, :n], ph[:, :n], AF.Relu, bias=bv[:, 6:7])
            hc = hcP[:, hi, :]
            nc.vector.tensor_scalar_add(hc[:, :n], hm[:, :n], C)
            if next_a is not None:
                next_a()
            # ---- z diff logits (pivot = u gate) ----
            z3 = pool3.tile([H, 3, MMT], f32, name="z3", tag="p3")
            for m in range(3):
                nc.tensor.matmul(z3[:, m, :n], wdlr[:, m], cpair8, start=True,
                                 stop=False, perf_mode=DR, skip_group_check=True)
                nc.tensor.matmul(z3[:, m, :n], wdu[:, m], ub[:, :n],
                                 start=False, stop=False, skip_group_check=True)
            for m in range(3):
                nc.tensor.matmul(z3[:, m, :n], wdh[:, m], hc[:, :n],
                                 start=False, stop=True, skip_group_check=True)
            fz = fzP[:, :, hi, :]
            if exp3_ok:
                nc.scalar.activation(fz[:, :, :n], z3[:, :, :n], AF.Exp,
                                     bias=bv[:, 3:4], scale=1.0 / SW)
            else:
                for m in range(3):
                    nc.scalar.activation(fz[:, m, :n], z3[:, m, :n], AF.Exp,
                                         bias=bv[:, 3 + m:4 + m], scale=1.0 / SW)
            if combine is not None:
                combine()

        def do_combine(cbv, BL, s, W, fzf, hcf, ubf, out_ap, out8_pair):
            """Combine over W nodes starting at tile base s (one tile or a
            pair flattened to [H, *, W] APs):
            out = (u + f0*hH + fL*hL + fR*hR)/(1+f0+fL+fR)."""
            t0, t1 = s // BL, (s + W) // BL
            cslL = cbv[:, 0, t0:t1, :]
            cslR = cbv[:, 1, t0:t1, :]
            mlr = spool.tile([H, 2, 2 * MMT], bf16, name="mlr", tag="mlr")
            mlrv = mlr[:].rearrange("p m (T b) -> p m T b", b=BL)
            fzv = fzf.rearrange("p m (T b) -> p m T b", b=BL)
            nc.vector.tensor_tensor(mlrv[:, 0, :t1 - t0, :], fzv[:, 1, :t1 - t0, :],
                                    cslL, OP.mult)
            nc.vector.tensor_tensor(mlrv[:, 1, :t1 - t0, :], fzv[:, 2, :t1 - t0, :],
                                    cslR, OP.mult)
            mS = spool.tile([H, 2 * MMT], bf16, name="mS", tag="mS")
            nc.vector.tensor_tensor(mS[:, :W], mlr[:, 0, :W], mlr[:, 1, :W], OP.add)
            m0 = spool.tile([H, 2 * MMT], bf16, name="m0", tag="m0")
            nc.vector.tensor_tensor(m0[:, :W], hcf[:, :W], fzf[:, 0, :W], OP.mult)
            a1 = spool.tile([H, 2 * MMT], bf16, name="a1", tag="a1")
            nc.vector.tensor_tensor(a1[:, :W], ubf[:, :W], m0[:, :W], OP.add)
            a2 = spool.tile([H, 2 * MMT], bf16, name="a2", tag="a2")
            nc.vector.tensor_tensor(a2[:, :W], a1[:, :W], mS[:, :W], OP.add)
            fS = spool.tile([H, 2 * MMT], bf16, name="fS", tag="fS")
            nc.vector.tensor_tensor(fS[:, :W], fzf[:, 1, :W], fzf[:, 2, :W], OP.add)
            den = spool.tile([H, 2 * MMT], f32, name="den", tag="den")
            nc.vector.scalar_tensor_tensor(den[:, :W], fzf[:, 0, :W], 1.0,
                                           fS[:, :W], OP.add, OP.add)
            rcp = spool.tile([H, 2 * MMT], f32, name="rcp", tag="rcp")
            nc.vector.reciprocal_approx_fast(rcp[:, :W], den[:, :W])
            nc.vector.tensor_tensor(out_ap, a2[:, :W], rcp[:, :W], OP.mult)
            if out8_pair is not None:
                o8src, o8dst = out8_pair
                nc.scalar.copy(o8dst, o8src)

        class Level:
            """One level of nj nodes with per-tree block size BL=2^j.
            Levels are stored per-tree bit-reversed, so children of the node
            at position q (tree-local) sit at child-buffer positions q and
            q+BL within the tree's 2*BL block; all writes are contiguous.
            prefetch_u() can be called from the PREVIOUS level's tail so the
            first u evictions enqueue ahead of its combine backlog."""

            def __init__(self, nj, BL, ct_base, cb, cb8, ob, ob8, ob_goff,
                         extra_work, tsz=MMT):
                self.nj, self.BL, self.ct_base = nj, BL, ct_base
                self.cb, self.cb8 = cb, cb8
                self.ob, self.ob8, self.ob_goff = ob, ob8, ob_goff
                self.extra_work = extra_work
                self.tiles = [(s, min(tsz, nj - s)) for s in range(0, nj, tsz)]
                self.ctts = {}
                self.us = {}
                self.ubps = {}

            def ct_ap(self, s, n):
                d = (s // 2048) * 2048
                if d not in self.ctts:
                    w = min(2048, self.nj - d)
                    t = ctpool.tile([FEAT + 1, 2048], bf16, name="ctt", tag="ctt")
                    nc.sync.dma_start(t[:, :w],
                                      ct_d[:, self.ct_base + d:self.ct_base + d + w])
                    self.ctts[d] = t
                return self.ctts[d][:, s - d:s - d + n]

            def do_u(self, t):
                if t < len(self.tiles) and t not in self.us:
                    s, n = self.tiles[t]
                    if t % 2 == 0:
                        self.ubps[t // 2] = upool.tile([H, 2, MMT], bf16,
                                                       name="ubP", tag="ubP")
                    self.us[t] = u_pass(self.ct_ap(s, n), n,
                                        self.ubps[t // 2], t % 2)

            def prefetch_u(self):
                self.do_u(0)
                self.do_u(1)

        def run_level(lv, next_lv=None):
            nj, BL = lv.nj, lv.BL
            ob, ob8, ob_goff = lv.ob, lv.ob8, lv.ob_goff
            extra_work = lv.extra_work
            cbv = lv.cb[:].rearrange("p (T two b) -> p two T b", two=2, b=BL)
            cb8v = lv.cb8[:].rearrange("p (T two b) -> p two T b", two=2, b=BL)
            tiles = lv.tiles
            us = lv.us
            ubps = lv.ubps
            do_u = lv.do_u

            rhs_ = {}

            def do_a(t):
                s, n = tiles[t]
                rhs_[t] = stage_a(cbv, cb8v, BL, s, n, us[t])

            do_u(0)
            do_u(1)
            do_a(0)
            fzP = hcP = None
            for t, (s, n) in enumerate(tiles):
                def next_a(t=t):
                    if t + 1 < len(tiles):
                        do_a(t + 1)
                    do_u(t + 2)
                    if t == max(0, len(tiles) - 3) and next_lv is not None:
                        next_lv.prefetch_u()
                    for _ in range(min(4, len(extra_work))):
                        extra_work.pop(0)()
                if t % 2 == 0:
                    fzP = fpool.tile([H, 3, 2, MMT], bf16, name="fzP", tag="fzP")
                    hcP = hpool.tile([H, 2, MMT], bf16, name="hcP", tag="hcP")
                # the last two tiles combine singly: the NEXT level's early
                # tiles depend on this level's final outputs (via the fp8
                # mirror), so halving the tail chain latency beats the pair
                # overhead saving there
                single = t >= len(tiles) - 2 or n < MMT
                if single or t % 2 == 1:
                    if single:
                        s0, W = s, n
                        hi = t % 2
                        fzf = fzP[:, :, hi, :]
                        hcf = hcP[:, hi, :]
                        ubf = ubps[t // 2][:, hi, :]
                    else:
                        s0 = tiles[t - 1][0]
                        W = s + n - s0
                        fzf = fzP[:].rearrange("p m two n -> p m (two n)")
                        hcf = hcP[:].rearrange("p two n -> p (two n)")
                        ubf = ubps[t // 2][:].rearrange("p two n -> p (two n)")
                    o_ap = ob[:, ob_goff + s0:ob_goff + s0 + W]
                    o8 = None if ob8 is None else \
                        (o_ap, ob8[:, ob_goff + s0:ob_goff + s0 + W])
                    combine = (lambda s0=s0, W=W, fzf=fzf, hcf=hcf, ubf=ubf,
                               o_ap=o_ap, o8=o8:
                               do_combine(cbv, BL, s0, W, fzf, hcf, ubf,
                                          o_ap, o8))
                else:
                    combine = None
                stage_b(cbv, cb8v, BL, s, n, us[t], rhs_[t],
                        fzP, hcP, t % 2, next_a, combine)

        # leaf (level 9) u-only tiles (bit-reversed ct order -> plain writes)
        def leaf_work(c):
            """Return list of closures, one per leaf tile of chunk c."""
            nleaf = TCH * 512
            base9 = LOFF[9] + c * nleaf
            work = []
            ctts = {}

            def ct_ap(s):
                d = (s // 2048) * 2048
                if d not in ctts:
                    t = ctpool.tile([FEAT + 1, 2048], bf16, name="ctt", tag="ctt")
                    nc.sync.dma_start(t[:], ct_d[:, base9 + d:base9 + d + 2048])
                    ctts[d] = t
                return ctts[d][:, s - d:s - d + MMT]

            def one(s):
                pu = poolPU.tile([H, MMT], f32, name="pu", tag="pu")
                nc.tensor.matmul(pu[:, :MMT], wu[:], ct_ap(s), start=True, stop=True)
                nc.vector.tensor_scalar(e9[:, s:s + MMT], pu[:, :MMT],
                                        0.0, C, OP.max, OP.add)
                nc.scalar.copy(e9m[:, s:s + MMT], e9[:, s:s + MMT])

            for s in range(0, nleaf, MMT):
                work.append(lambda s=s: one(s))
            return work

        # ================= phase A: chunks over levels 8,7,6 =================
        for w in leaf_work(0):
            w()
        # Build the full level schedule. Per chunk, order is
        #   ..., L8(c+1), L6(c), L7(c+1), ...
        # L6(c) and L8(c+1) touch disjoint buffers, so each short level sits
        # between independent work and level tails (whose final outputs gate
        # the consumer's early tiles) stay hidden. Leaf tiles of chunk c+1
        # drain through L7(c)'s and L8(c+1)'s next_a slots.
        def A8(c, work):
            return Level(TCH * 256, 256, LOFF[8] + c * TCH * 256,
                         e9, e9m, e8, e8m, 0, work)

        def A7(c, work):
            return Level(TCH * 128, 128, LOFF[7] + c * TCH * 128,
                         e8, e8m, e7, e7m, 0, work)

        def A6(c):
            return Level(TCH * 64, 64, LOFF[6] + c * TCH * 64,
                         e7, e7m, em6, em6m, c * TCH * 64, [])

        levels = []
        pend0 = leaf_work(0)
        for w in pend0[:4]:
            w()
        del pend0[:4]
        pend = leaf_work(1)
        levels.append(A8(0, pend0))
        levels.append(A7(0, pend))
        for c in range(NCHUNK - 1):
            nxt = leaf_work(c + 2) if c + 2 < NCHUNK else []
            levels.append(A8(c + 1, pend))
            levels.append(A6(c))
            levels.append(A7(c + 1, nxt))
            pend = nxt
        levels.append(A6(NCHUNK - 1))
        chain = [(em6, em6m, em5, em5m), (em5, em5m, e4, e4m),
                 (e4, e4m, e3, e3m), (e3, e3m, e2, e2m), (e2, e2m, e1, e1m)]
        for j, (cbuf, cb8, obuf, ob8) in zip(range(5, 0, -1), chain):
            # tail levels split into 4 sub-tiles: overlap is scarcest there
            tsz = MMT if TPC * 2 ** j > 2 * MMT else (TPC * 2 ** j) // 4
            levels.append(Level(TPC * 2 ** j, 2 ** j, LOFF[j],
                                cbuf, cb8, obuf, ob8, 0, [], tsz))
        levels.append(Level(TPC, 1, LOFF[0], e1, e1m, e0f, None, 0, [],
                            TPC // 4))

        for i, lv in enumerate(levels):
            nxt = levels[i + 1] if i + 1 < len(levels) else None
            run_level(lv, nxt)
            # safety: any leaf tiles not drained by the slots run now
            while lv.extra_work:
                lv.extra_work.pop(0)()

        # ================= output transpose + store =================
        pt = poolPH.tile([H, MMT], f32, name="ptr", tag="ph")
        nc.tensor.matmul(pt[:, :H], e0f[:], idt[:], is_transpose=True,
                         start=True, stop=True)
        osb = spool.tile([H, MMT], f32, name="osb", tag="osb")
        nc.vector.tensor_copy(osb[:, :H], pt[:, :H])
        nc.sync.dma_start(out_d[:], osb[:, :H])

    nc.compile()
    if not nc.is_finalized():
        nc.finalize()
    return nc


def _prepare(inputs):
    import ml_dtypes

    bf = ml_dtypes.bfloat16
    f8 = ml_dtypes.float8_e4m3

    contents = np.ascontiguousarray(np.asarray(inputs["contents"], np.float32))
    W_u = np.asarray(inputs["W_u"], np.float32)
    b_u = np.asarray(inputs["b_u"], np.float32)
    W_h = np.asarray(inputs["W_h"], np.float32)
    b_h = np.asarray(inputs["b_h"], np.float32)
    W_z = np.asarray(inputs["W_z"], np.float32)
    b_z = np.asarray(inputs["b_z"], np.float32)
    W_r = np.asarray(inputs["W_r"], np.float32)
    b_r = np.asarray(inputs["b_r"], np.float32)
    cw = float(np.asarray(inputs["conv_w"]).reshape(-1)[0])
    cb_ = float(np.asarray(inputs["conv_b"]).reshape(-1)[0])
    A = cw * cw
    C = cw * cb_ + cb_

    # per-core feature-major contents + ones row, level-major columns;
    # within each tree's level block, nodes in bit-reversed order so the
    # on-chip level buffers (position order) give contiguous children reads
    def bitrev_perm(j):
        p = np.arange(2 ** j, dtype=np.int64)
        r = np.zeros_like(p)
        for b in range(j):
            r = (r << 1) | ((p >> b) & 1)
        return r

    cts = np.empty((NCORES, FEAT + 1, NPC), np.float32)
    cts[:, FEAT, :] = 1.0
    col = 0
    for j in range(L):
        nn = 2 ** j
        n = TPC * nn
        blk = contents[OFF[j]:OFF[j + 1]].reshape(NCORES, TPC, nn, FEAT)
        blk = blk[:, :, bitrev_perm(j), :].reshape(NCORES, n, FEAT)
        cts[:, :FEAT, col:col + n] = blk.transpose(0, 2, 1)
        col += n

    # u-conv weights: pu = (A*cw*W_u)^T c + A*(cw*b_u + cb)
    wu = np.empty((FEAT + 1, H), np.float32)
    wu[:FEAT] = A * cw * W_u
    wu[FEAT] = A * (cw * b_u + cb_)

    # r weights (x SW, fp8): blocks rows 0:H=hL, H:2H=hR, 2H:3H=u
    wrlr = np.empty((H, 3, 2, H), np.float32)
    wru = np.empty((H, 3, H), np.float32)
    for m in range(3):
        blk = slice(m * H, (m + 1) * H)
        wrlr[:, m, 0, :] = SW * W_r[0:H, blk]
        wrlr[:, m, 1, :] = SW * W_r[H:2 * H, blk]
        wru[:, m, :] = SW * W_r[2 * H:3 * H, blk]

    # z diff weights: Wd_m = W_z[:, m] - W_z[:, u-gate], rows 0:H=hH, H:2H=hL,
    # 2H:3H=hR, 3H:4H=u
    wdlr = np.empty((H, 3, 2, H), np.float32)
    wdh = np.empty((H, 3, H), np.float32)
    wdu = np.empty((H, 3, H), np.float32)
    bd = np.empty((3, H), np.float32)
    for m in range(3):
        Wd = W_z[:, m * H:(m + 1) * H] - W_z[:, 3 * H:4 * H]
        bd[m] = b_z[m * H:(m + 1) * H] - b_z[3 * H:4 * H]
        wdlr[:, m, 0, :] = SW * Wd[H:2 * H]
        wdlr[:, m, 1, :] = SW * Wd[2 * H:3 * H]
        wdh[:, m, :] = SW * Wd[0:H]
        wdu[:, m, :] = SW * Wd[3 * H:4 * H]

    # h weights: ph = (0.5*A*cw*W_h)^T rh'  with rh' = (t+1)*hhu
    wh = np.ascontiguousarray((0.5 * A * cw * W_h).reshape(3, H, H).transpose(1, 0, 2))

    bvec = np.zeros((H, 8), np.float32)
    bvec[:, 0] = 0.5 * b_r[0:H]
    bvec[:, 1] = 0.5 * b_r[H:2 * H]
    bvec[:, 2] = 0.5 * b_r[2 * H:3 * H]
    bvec[:, 3] = bd[0]
    bvec[:, 4] = bd[1]
    bvec[:, 5] = bd[2]
    bvec[:, 6] = A * (cw * b_h + cb_)

    tanh3_ok = bool(np.array_equal(bvec[:, 0], bvec[:, 1])
                    and np.array_equal(bvec[:, 1], bvec[:, 2]))
    exp3_ok = bool(np.array_equal(bvec[:, 3], bvec[:, 4])
                   and np.array_equal(bvec[:, 4], bvec[:, 5]))

    common = {
        "wu": np.ascontiguousarray(wu).astype(bf),
        "wrlr": np.ascontiguousarray(wrlr).astype(f8),
        "wru": np.ascontiguousarray(wru).astype(bf),
        "wdlr": np.ascontiguousarray(wdlr).astype(f8),
        "wdh": np.ascontiguousarray(wdh).astype(bf),
        "wdu": np.ascontiguousarray(wdu).astype(bf),
        "wh": wh.astype(bf),
        "bvec": bvec,
        "ident": np.eye(H, dtype=np.float32),
    }
    in_maps = [dict(common, ct=np.ascontiguousarray(cts[c]).astype(bf))
               for c in range(NCORES)]
    return in_maps, tanh3_ok, exp3_ok


def kernel(**inputs):
    children = np.asarray(inputs["children"])
    cw = float(np.asarray(inputs["conv_w"]).reshape(-1)[0])
    cb_ = float(np.asarray(inputs["conv_b"]).reshape(-1)[0])
    collapsible = (cw >= 0.0) and (cb_ >= 0.0)
    if not _children_canonical(children) or not collapsible:
        args = {k: np.asarray(v) for k, v in inputs.items()}
        return _numpy_fallback(**args)

    from concourse.bass_utils import run_bass_kernel_spmd

    in_maps, tanh3_ok, exp3_ok = _prepare(inputs)
    key = (cw, cb_, tanh3_ok, exp3_ok)
    if key not in _CACHE:
        _CACHE[key] = _build(cw, cb_, tanh3_ok, exp3_ok)
    nc = _CACHE[key]

    res = run_bass_kernel_spmd(nc, in_maps, list(range(NCORES)))
    outs = [res.results[c]["out"] for c in range(NCORES)]
    return np.ascontiguousarray(np.concatenate(outs, axis=0).astype(np.float32))


if __name__ == "__main__":
    print("kernel_v3 module loaded")
